# revision 12
# baseline (speedup 1.0000x reference)
"""AnchorAttention distributed Bass kernel for 8 TRN2 NeuronCores.

Reference computation (B=2, S=4096, D=1024, H=16, Dh=64, A=512):
  anchors = x[:, :A];  queries = x[:, A:]
  anchor_q/k/v = split_heads(anchors @ Wq/Wk/Wv + b)
  query_q      = split_heads(queries @ Wqt + bqt)
  combined_q   = concat([anchor_q, query_q], axis=2)       # [B,H,S,Dh]
  out  = softmax(combined_q @ anchor_k^T / sqrt(Dh)) @ anchor_v
  out  = merge_heads(out) @ Wo + bo

Sharding: the B*S = 8192 token rows are split into 8 chunks of 1024 rows
(core c -> batch c//4, rows (c%4)*1024 ...). Each core duplicates its
batch's anchor K/V projections, computes Q for its own rows (Wq for the
anchor-region rows, Wqt for query rows), attention over the 512 anchors
for all 16 heads, and the output projection for its rows. The output is a
pure concatenation: no collectives.

Layout: everything is kept transposed ([feature, row]) so each matmul
contracts over the partition dim with zero on-chip transposes; the final
output projection naturally lands un-transposed [row, feature] for DMA
out. Host pre-transposes/pre-casts inputs to bf16 (compute dtype; f32
accumulation in PSUM). Softmax row-sums come free via an extra all-ones
column appended to V; no max-subtraction is needed (scores are ~N(0,1),
exp stays in a tiny range; softmax is shift-invariant so results match).

Heads are packed two per 128-partition tile (head h -> column-tile h//2,
partitions (h%2)*64 ..). The odd head of each pair has its V-slab ones
column *first* so the AV output [sums; attn^T] fits partitions 63..127.
"""

import numpy as np
import ml_dtypes

import concourse.bass as bass
import concourse.tile as tile
from concourse import bacc, mybir
from concourse import bass_utils

BF16 = mybir.dt.bfloat16
F32 = mybir.dt.float32

B, S, D = 2, 4096, 1024
H, DH = 16, 64
A = 512                  # num_anchor_tokens (asserted at runtime)
RPC = 1024               # rows per core
NCORES = 8
SCALE = 1.0 / np.sqrt(float(DH))

_CACHE = {}


def _build():
    """Build + compile the per-core Bass graph (identical on all cores)."""
    nc = bacc.Bacc("TRN2", target_bir_lowering=False, debug=False)

    xt = nc.dram_tensor("xt", [D, RPC], BF16, kind="ExternalInput")     # rows^T
    at = nc.dram_tensor("at", [D, A], BF16, kind="ExternalInput")       # anchors^T
    wlo = nc.dram_tensor("wlo", [D, D], BF16, kind="ExternalInput")     # Q weight rows 0-511
    whi = nc.dram_tensor("whi", [D, D], BF16, kind="ExternalInput")     # Q weight rows 512-1023
    wk = nc.dram_tensor("wk", [D, D], BF16, kind="ExternalInput")
    wv = nc.dram_tensor("wv", [D, D], BF16, kind="ExternalInput")
    wo = nc.dram_tensor("wo", [D, D], BF16, kind="ExternalInput")
    blo = nc.dram_tensor("blo", [D], F32, kind="ExternalInput")
    bhi = nc.dram_tensor("bhi", [D], F32, kind="ExternalInput")
    bk = nc.dram_tensor("bk", [D], F32, kind="ExternalInput")
    bv = nc.dram_tensor("bv", [D], F32, kind="ExternalInput")
    bo = nc.dram_tensor("bo", [D], F32, kind="ExternalInput")
    out = nc.dram_tensor("out", [RPC, D], F32, kind="ExternalOutput")

    Exp = mybir.ActivationFunctionType.Exp

    with tile.TileContext(nc) as tc:
        with tc.tile_pool(name="wpool", bufs=1) as wpool, \
             tc.tile_pool(name="xpool", bufs=1) as xpool, \
             tc.tile_pool(name="cpool", bufs=1) as cpool, \
             tc.tile_pool(name="kvpool", bufs=1) as kvpool, \
             tc.tile_pool(name="qtpool", bufs=2) as qtpool, \
             tc.tile_pool(name="ptpool", bufs=3) as ptpool, \
             tc.tile_pool(name="tmppool", bufs=3) as tmppool, \
             tc.tile_pool(name="rcppool", bufs=2) as rcppool, \
             tc.tile_pool(name="attnpool", bufs=1) as attnpool, \
             tc.tile_pool(name="outpool", bufs=3) as outpool, \
             tc.tile_pool(name="psum", bufs=2, space="PSUM") as psum:

            # ---- input DMAs (slab layouts: partition = dim%128) ----
            def slab_in(pool, t, cols, name):
                s = pool.tile([128, 8, cols], BF16, name=name)
                nc.sync.dma_start(
                    out=s, in_=t.ap().rearrange("(dt p) c -> p dt c", p=128))
                return s

            wk_sb = slab_in(wpool, wk, D, "wk_sb")
            at_sb = slab_in(xpool, at, A, "at_sb")
            wv_sb = slab_in(wpool, wv, D, "wv_sb")
            xt_sb = slab_in(xpool, xt, RPC, "xt_sb")
            wlo_sb = slab_in(wpool, wlo, D, "wlo_sb")
            whi_sb = slab_in(wpool, whi, D, "whi_sb")
            wo_sb = slab_in(wpool, wo, D, "wo_sb")

            def bias_in(t, name):  # [D] -> [128, 8] (partition = c%128, col = c//128)
                s = cpool.tile([128, 8], F32, name=name)
                nc.sync.dma_start(
                    out=s, in_=t.ap().rearrange("(ct p) -> p ct", p=128))
                return s

            blo_sb = bias_in(blo, "blo_sb")
            bhi_sb = bias_in(bhi, "bhi_sb")
            bk_sb = bias_in(bk, "bk_sb")

            def bias_bc(t, name):  # [D] -> broadcast to [128, D]
                s = cpool.tile([128, D], F32, name=name)
                a = t.ap()
                nc.sync.dma_start(
                    out=s,
                    in_=bass.AP(tensor=a.tensor, offset=a.offset,
                                ap=[[0, 128]] + list(a.ap)))
                return s

            bv_bc = bias_bc(bv, "bv_bc")
            bo_bc = bias_bc(bo, "bo_bc")



            # V slab: [128(a%128), ach, head, 65]; cols 0-63 = V head slice,
            # col 64 = ones (supplies softmax row-sums during AV).
            vaug = kvpool.tile([128, 4, H, DH + 1], BF16, name="vaug")
            nc.vector.memset(vaug, 1.0)

            # ---- K^T projection: kt[c, a] = (anchors @ Wk)^T ----
            kt_sb = kvpool.tile([128, 8, A], BF16, name="kt_sb")
            for ct in range(8):
                pk = psum.tile([128, A], F32, tag="proj", name="pk")
                for dt in range(8):
                    nc.tensor.matmul(
                        pk, wk_sb[:, dt, ct * 128:(ct + 1) * 128],
                        at_sb[:, dt, :], start=(dt == 0), stop=(dt == 7))
                nc.vector.tensor_scalar_add(
                    kt_sb[:, ct, :], pk, bk_sb[:, ct:ct + 1])

            # ---- V projection (un-transposed): v[a, c] = anchors @ Wv ----
            for ach in range(4):
                for ch in range(2):
                    pv = psum.tile([128, 512], F32, tag="proj", name="pv")
                    for dt in range(8):
                        nc.tensor.matmul(
                            pv, at_sb[:, dt, ach * 128:(ach + 1) * 128],
                            wv_sb[:, dt, ch * 512:(ch + 1) * 512],
                            start=(dt == 0), stop=(dt == 7))
                    # scatter heads ch*8..ch*8+7 into vaug slots
                    pv_v = pv.rearrange("p (hd d) -> p hd d", d=DH)
                    bv_v = bv_bc.rearrange("p (chd hd d) -> p chd hd d",
                                           chd=2, d=DH)[:, ch]
                    nc.vector.tensor_add(
                        vaug[:, ach, ch * 8:(ch + 1) * 8, 0:DH],
                        pv_v, bv_v)

            # ---- Q^T projection per 512-row chunk ----
            qts = []
            for rc in range(2):
                wsel = wlo_sb if rc == 0 else whi_sb
                bsel = blo_sb if rc == 0 else bhi_sb
                qt_sb = qtpool.tile([128, 8, 512], BF16, tag="qt",
                                    name=f"qt_sb{rc}")
                for ct in range(8):
                    pq = psum.tile([128, 512], F32, tag="proj", name="pq")
                    for dt in range(8):
                        nc.tensor.matmul(
                            pq, wsel[:, dt, ct * 128:(ct + 1) * 128],
                            xt_sb[:, dt, rc * 512:(rc + 1) * 512],
                            start=(dt == 0), stop=(dt == 7))
                    nc.vector.tensor_scalar_add(
                        qt_sb[:, ct, :], pq, bsel[:, ct:ct + 1])
                qts.append(qt_sb)

            # ---- attention + output projection, per 512-row chunk ----
            attnT = attnpool.tile([128, 8, RPC], BF16, name="attnT")
            for rc in range(2):
                qt_sb = qts[rc]
                for h in range(H):
                    ct, par = h // 2, h % 2
                    po = par * 64          # partition offset of this head
                    # scores S^T[a, r] = K @ Q^T, per 128-anchor chunk
                    pt = ptpool.tile([128, 4, 512], BF16, tag="pt", name="pt")
                    for ach in range(4):
                        ps_s = psum.tile([128, 512], F32, tag="s", name="ps_s",
                                         bufs=3)
                        nc.tensor.matmul(
                            ps_s, kt_sb[po:po + 64, ct, ach * 128:(ach + 1) * 128],
                            qt_sb[po:po + 64, ct, :], start=True, stop=True)
                        nc.scalar.activation(
                            out=pt[:, ach, :], in_=ps_s, func=Exp, scale=SCALE)
                    # AV: [attn^T; sums] via ones-augmented V, rows 0-64
                    pav = psum.tile([128, 512], F32, tag="attn", name="pav",
                                    bufs=3)
                    for ach in range(4):
                        nc.tensor.matmul(
                            pav[0:DH + 1, :], vaug[:, ach, h, :],
                            pt[:, ach, :], start=(ach == 0), stop=(ach == 3))
                    rcp = rcppool.tile([1, 512], F32, tag="rcp", name="rcp")
                    nc.vector.reciprocal(rcp, pav[DH:DH + 1, :])
                    # partition-broadcast 1/sums to 64 rows via DMA
                    # (free-dim stride-0 source; partition stride-0 is
                    # rejected by AP validation)
                    bc_t = tmppool.tile([DH, 512], F32, tag="bc", name="bc_t")
                    nc.sync.dma_start(
                        out=bc_t,
                        in_=bass.AP(tensor=rcp.tensor, offset=rcp.offset,
                                    ap=[list(rcp.ap)[0], [0, DH]]
                                    + list(rcp.ap)[1:]))
                    if par == 0:
                        nc.vector.tensor_mul(
                            attnT[0:DH, ct, rc * 512:(rc + 1) * 512],
                            pav[0:DH, :], bc_t)
                    else:
                        # odd heads live at partitions 64-127 of the slab;
                        # DVE can't cross partitions, so stage + DMA-shift.
                        stg = tmppool.tile([DH, 512], BF16, tag="stg",
                                           name="stg")
                        nc.vector.tensor_mul(stg, pav[0:DH, :], bc_t)
                        nc.sync.dma_start(
                            out=attnT[DH:2 * DH, ct, rc * 512:(rc + 1) * 512],
                            in_=stg)

                # ---- output projection for this row chunk ----
                for rt in range(4):
                    rti = rc * 4 + rt
                    for nh in range(2):
                        pout = psum.tile([128, 512], F32, tag="proj",
                                         name="pout")
                        for ct2 in range(8):
                            nc.tensor.matmul(
                                pout, attnT[:, ct2, rti * 128:(rti + 1) * 128],
                                wo_sb[:, ct2, nh * 512:(nh + 1) * 512],
                                start=(ct2 == 0), stop=(ct2 == 7))
                        out_t = outpool.tile([128, 512], F32, tag="out",
                                             name="out_t")
                        nc.vector.tensor_add(out_t, pout,
                                             bo_bc[:, nh * 512:(nh + 1) * 512])
                        nc.sync.dma_start(
                            out=out.ap()[rti * 128:(rti + 1) * 128,
                                         nh * 512:(nh + 1) * 512],
                            in_=out_t)

    nc.compile()
    return nc


def _make_in_maps(x, Wq, bq, Wk, bk, Wv, bv, Wqt, bqt, Wo, bo):
    x = np.asarray(x, dtype=np.float32)
    bf = ml_dtypes.bfloat16

    wq_b = np.ascontiguousarray(np.asarray(Wq, np.float32).astype(bf))
    wqt_b = np.ascontiguousarray(np.asarray(Wqt, np.float32).astype(bf))
    wk_b = np.ascontiguousarray(np.asarray(Wk, np.float32).astype(bf))
    wv_b = np.ascontiguousarray(np.asarray(Wv, np.float32).astype(bf))
    wo_b = np.ascontiguousarray(np.asarray(Wo, np.float32).astype(bf))
    f32 = lambda v: np.ascontiguousarray(np.asarray(v, np.float32))
    bq, bqt, bk, bv, bo = map(f32, (bq, bqt, bk, bv, bo))

    in_maps = []
    for c in range(NCORES):
        b, q = divmod(c, 4)
        rows = x[b, q * RPC:(q + 1) * RPC, :]
        in_maps.append({
            "xt": np.ascontiguousarray(rows.T.astype(bf)),
            "at": np.ascontiguousarray(x[b, :A, :].T.astype(bf)),
            "wlo": wq_b if q == 0 else wqt_b,
            "whi": wqt_b,
            "wk": wk_b, "wv": wv_b, "wo": wo_b,
            "blo": bq if q == 0 else bqt, "bhi": bqt,
            "bk": bk, "bv": bv, "bo": bo,
        })
    return in_maps


def kernel(x, Wq, bq, Wk, bk, Wv, bv, Wqt, bqt, Wo, bo, num_anchor_tokens):
    assert int(num_anchor_tokens) == A
    if "nc" not in _CACHE:
        _CACHE["nc"] = _build()
    nc = _CACHE["nc"]

    in_maps = _make_in_maps(x, Wq, bq, Wk, bk, Wv, bv, Wqt, bqt, Wo, bo)
    res = bass_utils.run_bass_kernel_spmd(
        nc, in_maps, core_ids=list(range(NCORES)))
    out = np.empty((B, S, D), np.float32)
    for c in range(NCORES):
        b, q = divmod(c, 4)
        out[b, q * RPC:(q + 1) * RPC, :] = res.results[c]["out"]
    return out


# revision 16
# speedup vs baseline: 1.0026x; 1.0026x over previous
"""AnchorAttention distributed Bass kernel for 8 TRN2 NeuronCores.

Reference computation (B=2, S=4096, D=1024, H=16, Dh=64, A=512):
  anchors = x[:, :A];  queries = x[:, A:]
  anchor_q/k/v = split_heads(anchors @ Wq/Wk/Wv + b)
  query_q      = split_heads(queries @ Wqt + bqt)
  combined_q   = concat([anchor_q, query_q], axis=2)       # [B,H,S,Dh]
  out  = softmax(combined_q @ anchor_k^T / sqrt(Dh)) @ anchor_v
  out  = merge_heads(out) @ Wo + bo

Sharding: the B*S = 8192 token rows are split into 8 chunks of 1024 rows
(core c -> batch c//4, rows (c%4)*1024 ...). Each core duplicates its
batch's anchor K/V projections, computes Q for its own rows (Wq for the
anchor-region rows, Wqt for query rows), attention over the 512 anchors
for all 16 heads, and the output projection for its rows. The output is a
pure concatenation: no collectives.

Layout: everything is kept transposed ([feature, row]) so each matmul
contracts over the partition dim with zero on-chip transposes; the final
output projection naturally lands un-transposed [row, feature] for DMA
out. Host pre-transposes/pre-casts inputs to bf16 (compute dtype; f32
accumulation in PSUM). Softmax row-sums come free via an extra all-ones
column appended to V; no max-subtraction is needed (scores are ~N(0,1),
exp stays in a tiny range; softmax is shift-invariant so results match).

Heads are packed two per 128-partition tile (head h -> column-tile h//2,
partitions (h%2)*64 ..). The odd head of each pair has its V-slab ones
column *first* so the AV output [sums; attn^T] fits partitions 63..127.
"""

import numpy as np
import ml_dtypes

import concourse.bass as bass
import concourse.tile as tile
from concourse import bacc, mybir
from concourse import bass_utils

BF16 = mybir.dt.bfloat16
F32 = mybir.dt.float32

B, S, D = 2, 4096, 1024
H, DH = 16, 64
A = 512                  # num_anchor_tokens (asserted at runtime)
RPC = 1024               # rows per core
NCORES = 8
SCALE = 1.0 / np.sqrt(float(DH))

_CACHE = {}


def _build():
    """Build + compile the per-core Bass graph (identical on all cores)."""
    nc = bacc.Bacc("TRN2", target_bir_lowering=False, debug=False)

    xt = nc.dram_tensor("xt", [D, RPC], BF16, kind="ExternalInput")     # rows^T
    at = nc.dram_tensor("at", [D, A], BF16, kind="ExternalInput")       # anchors^T
    wlo = nc.dram_tensor("wlo", [D, D], BF16, kind="ExternalInput")     # Q weight rows 0-511
    whi = nc.dram_tensor("whi", [D, D], BF16, kind="ExternalInput")     # Q weight rows 512-1023
    wk = nc.dram_tensor("wk", [D, D], BF16, kind="ExternalInput")
    wv = nc.dram_tensor("wv", [D, D], BF16, kind="ExternalInput")
    wo = nc.dram_tensor("wo", [D, D], BF16, kind="ExternalInput")
    blo = nc.dram_tensor("blo", [D], F32, kind="ExternalInput")
    bhi = nc.dram_tensor("bhi", [D], F32, kind="ExternalInput")
    bk = nc.dram_tensor("bk", [D], F32, kind="ExternalInput")
    bv = nc.dram_tensor("bv", [D], F32, kind="ExternalInput")
    bo = nc.dram_tensor("bo", [D], F32, kind="ExternalInput")
    out = nc.dram_tensor("out", [RPC, D], F32, kind="ExternalOutput")

    Exp = mybir.ActivationFunctionType.Exp

    with tile.TileContext(nc) as tc:
        with tc.tile_pool(name="wpool", bufs=1) as wpool, \
             tc.tile_pool(name="xpool", bufs=1) as xpool, \
             tc.tile_pool(name="cpool", bufs=1) as cpool, \
             tc.tile_pool(name="kvpool", bufs=1) as kvpool, \
             tc.tile_pool(name="qtpool", bufs=2) as qtpool, \
             tc.tile_pool(name="ptpool", bufs=3) as ptpool, \
             tc.tile_pool(name="tmppool", bufs=3) as tmppool, \
             tc.tile_pool(name="rcppool", bufs=2) as rcppool, \
             tc.tile_pool(name="attnpool", bufs=1) as attnpool, \
             tc.tile_pool(name="outpool", bufs=3) as outpool, \
             tc.tile_pool(name="psum", bufs=2, space="PSUM") as psum:

            # ---- input DMAs (slab layouts: partition = dim%128) ----
            def slab_in(pool, t, cols, name):
                s = pool.tile([128, 8, cols], BF16, name=name)
                nc.sync.dma_start(
                    out=s, in_=t.ap().rearrange("(dt p) c -> p dt c", p=128))
                return s

            wk_sb = slab_in(wpool, wk, D, "wk_sb")
            at_sb = slab_in(xpool, at, A, "at_sb")
            wv_sb = slab_in(wpool, wv, D, "wv_sb")
            xt_sb = slab_in(xpool, xt, RPC, "xt_sb")
            wlo_sb = slab_in(wpool, wlo, D, "wlo_sb")
            whi_sb = slab_in(wpool, whi, D, "whi_sb")
            wo_sb = slab_in(wpool, wo, D, "wo_sb")

            def bias_in(t, name):  # [D] -> [128, 8] (partition = c%128, col = c//128)
                s = cpool.tile([128, 8], F32, name=name)
                nc.sync.dma_start(
                    out=s, in_=t.ap().rearrange("(ct p) -> p ct", p=128))
                return s

            blo_sb = bias_in(blo, "blo_sb")
            bhi_sb = bias_in(bhi, "bhi_sb")
            bk_sb = bias_in(bk, "bk_sb")

            def bias_bc(t, name):  # [D] -> broadcast to [128, D]
                s = cpool.tile([128, D], F32, name=name)
                a = t.ap()
                nc.sync.dma_start(
                    out=s,
                    in_=bass.AP(tensor=a.tensor, offset=a.offset,
                                ap=[[0, 128]] + list(a.ap)))
                return s

            bv_bc = bias_bc(bv, "bv_bc")
            bo_bc = bias_bc(bo, "bo_bc")



            # V slab: [128(a%128), ach, head, 65]; cols 0-63 = V head slice,
            # col 64 = ones (supplies softmax row-sums during AV).
            vaug = kvpool.tile([128, 4, H, DH + 1], BF16, name="vaug")
            nc.vector.memset(vaug, 1.0)

            # ---- K^T projection: kt[c, a] = (anchors @ Wk)^T ----
            kt_sb = kvpool.tile([128, 8, A], BF16, name="kt_sb")
            for ct in range(8):
                pk = psum.tile([128, A], F32, tag="proj", name="pk")
                for dt in range(8):
                    nc.tensor.matmul(
                        pk, wk_sb[:, dt, ct * 128:(ct + 1) * 128],
                        at_sb[:, dt, :], start=(dt == 0), stop=(dt == 7))
                nc.vector.tensor_scalar_add(
                    kt_sb[:, ct, :], pk, bk_sb[:, ct:ct + 1])

            # ---- V projection (un-transposed): v[a, c] = anchors @ Wv ----
            for ach in range(4):
                for ch in range(2):
                    pv = psum.tile([128, 512], F32, tag="proj", name="pv")
                    for dt in range(8):
                        nc.tensor.matmul(
                            pv, at_sb[:, dt, ach * 128:(ach + 1) * 128],
                            wv_sb[:, dt, ch * 512:(ch + 1) * 512],
                            start=(dt == 0), stop=(dt == 7))
                    # scatter heads ch*8..ch*8+7 into vaug slots
                    pv_v = pv.rearrange("p (hd d) -> p hd d", d=DH)
                    bv_v = bv_bc.rearrange("p (chd hd d) -> p chd hd d",
                                           chd=2, d=DH)[:, ch]
                    nc.vector.tensor_add(
                        vaug[:, ach, ch * 8:(ch + 1) * 8, 0:DH],
                        pv_v, bv_v)

            # ---- Q^T projection per 512-row chunk ----
            qts = []
            for rc in range(2):
                wsel = wlo_sb if rc == 0 else whi_sb
                bsel = blo_sb if rc == 0 else bhi_sb
                qt_sb = qtpool.tile([128, 8, 512], BF16, tag="qt",
                                    name=f"qt_sb{rc}")
                for ct in range(8):
                    pq = psum.tile([128, 512], F32, tag="proj", name="pq")
                    for dt in range(8):
                        nc.tensor.matmul(
                            pq, wsel[:, dt, ct * 128:(ct + 1) * 128],
                            xt_sb[:, dt, rc * 512:(rc + 1) * 512],
                            start=(dt == 0), stop=(dt == 7))
                    nc.vector.tensor_scalar_add(
                        qt_sb[:, ct, :], pq, bsel[:, ct:ct + 1])
                qts.append(qt_sb)

            # ---- attention + output projection, per 512-row chunk ----
            attnT = attnpool.tile([128, 8, RPC], BF16, name="attnT")
            for rc in range(2):
                qt_sb = qts[rc]
                for ct in range(8):        # head pair (2*ct, 2*ct+1)
                    for par in range(2):
                        h = 2 * ct + par
                        po = par * 64      # partition offset of this head
                        # scores S^T[a, r] = K @ Q^T, per 128-anchor chunk
                        pt = ptpool.tile([128, 4, 512], BF16, tag="pt",
                                         name="pt")
                        for ach in range(4):
                            ps_s = psum.tile([128, 512], F32, tag="s",
                                             name="ps_s", bufs=4)
                            nc.tensor.matmul(
                                ps_s,
                                kt_sb[po:po + 64, ct, ach * 128:(ach + 1) * 128],
                                qt_sb[po:po + 64, ct, :], start=True, stop=True)
                            nc.scalar.activation(
                                out=pt[:, ach, :], in_=ps_s, func=Exp,
                                scale=SCALE)
                        # AV: [attn^T; sums] via ones-augmented V, rows 0-64
                        pav = psum.tile([128, 512], F32, tag="attn",
                                        name="pav", bufs=2)
                        for ach in range(4):
                            nc.tensor.matmul(
                                pav[0:DH + 1, :], vaug[:, ach, h, :],
                                pt[:, ach, :], start=(ach == 0),
                                stop=(ach == 3))
                        # normalization: sums row to partition 0 (ACT can
                        # cross partitions on single-partition copies), then
                        # fast reciprocal (needs partition-0 SBUF input),
                        # partition-broadcast via free-dim stride-0 DMA,
                        # multiply out of PSUM.
                        srow = rcppool.tile([1, 512], F32, tag="srow",
                                            name="srow")
                        nc.scalar.activation(
                            srow, pav[DH:DH + 1, :],
                            mybir.ActivationFunctionType.Copy)
                        rcp1 = rcppool.tile([1, 512], F32, tag="rcp",
                                            name="rcp1")
                        nc.vector.reciprocal_approx_fast(rcp1, srow)
                        bc_t = tmppool.tile([DH, 512], F32, tag="bc",
                                            name="bc_t")
                        nc.sync.dma_start(
                            out=bc_t,
                            in_=bass.AP(tensor=rcp1.tensor, offset=rcp1.offset,
                                        ap=[list(rcp1.ap)[0], [0, DH]]
                                        + list(rcp1.ap)[1:]))
                        if par == 0:
                            nc.vector.tensor_mul(
                                attnT[0:DH, ct, rc * 512:(rc + 1) * 512],
                                pav[0:DH, :], bc_t)
                        else:
                            # odd heads live at partitions 64-127 of the
                            # slab; DVE can't cross partitions, so stage +
                            # DMA-shift.
                            stg = tmppool.tile([DH, 512], BF16, tag="stg",
                                               name="stg")
                            nc.vector.tensor_mul(stg, pav[0:DH, :], bc_t)
                            nc.sync.dma_start(
                                out=attnT[DH:2 * DH, ct,
                                          rc * 512:(rc + 1) * 512],
                                in_=stg)

                # ---- output projection for this row chunk ----
                for rt in range(4):
                    rti = rc * 4 + rt
                    for nh in range(2):
                        pout = psum.tile([128, 512], F32, tag="proj",
                                         name="pout")
                        for ct2 in range(8):
                            nc.tensor.matmul(
                                pout, attnT[:, ct2, rti * 128:(rti + 1) * 128],
                                wo_sb[:, ct2, nh * 512:(nh + 1) * 512],
                                start=(ct2 == 0), stop=(ct2 == 7))
                        out_t = outpool.tile([128, 512], F32, tag="out",
                                             name="out_t")
                        nc.vector.tensor_add(out_t, pout,
                                             bo_bc[:, nh * 512:(nh + 1) * 512])
                        nc.sync.dma_start(
                            out=out.ap()[rti * 128:(rti + 1) * 128,
                                         nh * 512:(nh + 1) * 512],
                            in_=out_t)

    nc.compile()
    return nc


def _make_in_maps(x, Wq, bq, Wk, bk, Wv, bv, Wqt, bqt, Wo, bo):
    x = np.asarray(x, dtype=np.float32)
    bf = ml_dtypes.bfloat16

    wq_b = np.ascontiguousarray(np.asarray(Wq, np.float32).astype(bf))
    wqt_b = np.ascontiguousarray(np.asarray(Wqt, np.float32).astype(bf))
    wk_b = np.ascontiguousarray(np.asarray(Wk, np.float32).astype(bf))
    wv_b = np.ascontiguousarray(np.asarray(Wv, np.float32).astype(bf))
    wo_b = np.ascontiguousarray(np.asarray(Wo, np.float32).astype(bf))
    f32 = lambda v: np.ascontiguousarray(np.asarray(v, np.float32))
    bq, bqt, bk, bv, bo = map(f32, (bq, bqt, bk, bv, bo))

    in_maps = []
    for c in range(NCORES):
        b, q = divmod(c, 4)
        rows = x[b, q * RPC:(q + 1) * RPC, :]
        in_maps.append({
            "xt": np.ascontiguousarray(rows.T.astype(bf)),
            "at": np.ascontiguousarray(x[b, :A, :].T.astype(bf)),
            "wlo": wq_b if q == 0 else wqt_b,
            "whi": wqt_b,
            "wk": wk_b, "wv": wv_b, "wo": wo_b,
            "blo": bq if q == 0 else bqt, "bhi": bqt,
            "bk": bk, "bv": bv, "bo": bo,
        })
    return in_maps


def kernel(x, Wq, bq, Wk, bk, Wv, bv, Wqt, bqt, Wo, bo, num_anchor_tokens):
    assert int(num_anchor_tokens) == A
    if "nc" not in _CACHE:
        _CACHE["nc"] = _build()
    nc = _CACHE["nc"]

    in_maps = _make_in_maps(x, Wq, bq, Wk, bk, Wv, bv, Wqt, bqt, Wo, bo)
    res = bass_utils.run_bass_kernel_spmd(
        nc, in_maps, core_ids=list(range(NCORES)))
    out = np.empty((B, S, D), np.float32)
    for c in range(NCORES):
        b, q = divmod(c, 4)
        out[b, q * RPC:(q + 1) * RPC, :] = res.results[c]["out"]
    return out


# revision 17
# speedup vs baseline: 1.1706x; 1.1676x over previous
"""AnchorAttention distributed Bass kernel for 8 TRN2 NeuronCores.

Reference computation (B=2, S=4096, D=1024, H=16, Dh=64, A=512):
  anchors = x[:, :A];  queries = x[:, A:]
  anchor_q/k/v = split_heads(anchors @ Wq/Wk/Wv + b)
  query_q      = split_heads(queries @ Wqt + bqt)
  combined_q   = concat([anchor_q, query_q], axis=2)       # [B,H,S,Dh]
  out  = softmax(combined_q @ anchor_k^T / sqrt(Dh)) @ anchor_v
  out  = merge_heads(out) @ Wo + bo

Sharding: the B*S = 8192 token rows are split into 8 chunks of 1024 rows
(core c -> batch c//4, rows (c%4)*1024 ...). Each core duplicates its
batch's anchor K/V projections, computes Q for its own rows (Wq for the
anchor-region rows, Wqt for query rows), attention over the 512 anchors
for all 16 heads, and the output projection for its rows. The output is a
pure concatenation: no collectives.

Layout: everything is kept transposed ([feature, row]) so each matmul
contracts over the partition dim with zero on-chip transposes; the final
output projection naturally lands un-transposed [row, feature] for DMA
out. Host pre-transposes/pre-casts inputs to bf16 (compute dtype; f32
accumulation in PSUM). Softmax row-sums come free via an extra all-ones
column appended to V; no max-subtraction is needed (scores are ~N(0,1),
exp stays in a tiny range; softmax is shift-invariant so results match).

Heads are packed two per 128-partition tile (head h -> column-tile h//2,
partitions (h%2)*64 ..). The odd head of each pair has its V-slab ones
column *first* so the AV output [sums; attn^T] fits partitions 63..127.
"""

import numpy as np
import ml_dtypes

import concourse.bass as bass
import concourse.tile as tile
from concourse import bacc, mybir
from concourse import bass_utils

BF16 = mybir.dt.bfloat16
F32 = mybir.dt.float32

B, S, D = 2, 4096, 1024
H, DH = 16, 64
A = 512                  # num_anchor_tokens (asserted at runtime)
RPC = 1024               # rows per core
NCORES = 8
SCALE = 1.0 / np.sqrt(float(DH))

_CACHE = {}


def _build():
    """Build + compile the per-core Bass graph (identical on all cores)."""
    nc = bacc.Bacc("TRN2", target_bir_lowering=False, debug=False)

    xt = nc.dram_tensor("xt", [D, RPC], BF16, kind="ExternalInput")     # rows^T
    at = nc.dram_tensor("at", [D, A], BF16, kind="ExternalInput")       # anchors^T
    wlo = nc.dram_tensor("wlo", [D, D], BF16, kind="ExternalInput")     # Q weight rows 0-511
    whi = nc.dram_tensor("whi", [D, D], BF16, kind="ExternalInput")     # Q weight rows 512-1023
    wk = nc.dram_tensor("wk", [D, D], BF16, kind="ExternalInput")
    wv = nc.dram_tensor("wv", [D, D], BF16, kind="ExternalInput")
    wo = nc.dram_tensor("wo", [D, D], BF16, kind="ExternalInput")
    blo = nc.dram_tensor("blo", [D], F32, kind="ExternalInput")
    bhi = nc.dram_tensor("bhi", [D], F32, kind="ExternalInput")
    bk = nc.dram_tensor("bk", [D], F32, kind="ExternalInput")
    bv = nc.dram_tensor("bv", [D], F32, kind="ExternalInput")
    bo = nc.dram_tensor("bo", [D], F32, kind="ExternalInput")
    out = nc.dram_tensor("out", [RPC, D], F32, kind="ExternalOutput")

    Exp = mybir.ActivationFunctionType.Exp

    with tile.TileContext(nc) as tc:
        with tc.tile_pool(name="wpool", bufs=1) as wpool, \
             tc.tile_pool(name="xpool", bufs=1) as xpool, \
             tc.tile_pool(name="cpool", bufs=1) as cpool, \
             tc.tile_pool(name="kvpool", bufs=1) as kvpool, \
             tc.tile_pool(name="qtpool", bufs=2) as qtpool, \
             tc.tile_pool(name="ptpool", bufs=3) as ptpool, \
             tc.tile_pool(name="tmppool", bufs=3) as tmppool, \
             tc.tile_pool(name="rcppool", bufs=2) as rcppool, \
             tc.tile_pool(name="attnpool", bufs=1) as attnpool, \
             tc.tile_pool(name="outpool", bufs=3) as outpool, \
             tc.tile_pool(name="psum", bufs=2, space="PSUM") as psum:

            # ---- input DMAs (slab layouts: partition = dim%128) ----
            def slab_in(pool, t, cols, name):
                s = pool.tile([128, 8, cols], BF16, name=name)
                nc.sync.dma_start(
                    out=s, in_=t.ap().rearrange("(dt p) c -> p dt c", p=128))
                return s

            wk_sb = slab_in(wpool, wk, D, "wk_sb")
            at_sb = slab_in(xpool, at, A, "at_sb")
            wv_sb = slab_in(wpool, wv, D, "wv_sb")
            xt_sb = slab_in(xpool, xt, RPC, "xt_sb")
            wlo_sb = slab_in(wpool, wlo, D, "wlo_sb")
            whi_sb = slab_in(wpool, whi, D, "whi_sb")
            wo_sb = slab_in(wpool, wo, D, "wo_sb")

            def bias_in(t, name):  # [D] -> [128, 8] (partition = c%128, col = c//128)
                s = cpool.tile([128, 8], F32, name=name)
                nc.sync.dma_start(
                    out=s, in_=t.ap().rearrange("(ct p) -> p ct", p=128))
                return s

            blo_sb = bias_in(blo, "blo_sb")
            bhi_sb = bias_in(bhi, "bhi_sb")
            bk_sb = bias_in(bk, "bk_sb")

            def bias_bc(t, name):  # [D] -> broadcast to [128, D]
                s = cpool.tile([128, D], F32, name=name)
                a = t.ap()
                nc.sync.dma_start(
                    out=s,
                    in_=bass.AP(tensor=a.tensor, offset=a.offset,
                                ap=[[0, 128]] + list(a.ap)))
                return s

            bv_bc = bias_bc(bv, "bv_bc")
            bo_bc = bias_bc(bo, "bo_bc")



            # V slab: [128(a%128), ach, head, 65]; cols 0-63 = V head slice,
            # col 64 = ones (supplies softmax row-sums during AV).
            vaug = kvpool.tile([128, 4, H, DH + 1], BF16, name="vaug")
            nc.vector.memset(vaug, 1.0)

            # ---- K^T projection: kt[c, a] = (anchors @ Wk)^T ----
            kt_sb = kvpool.tile([128, 8, A], BF16, name="kt_sb")
            for ct in range(8):
                pk = psum.tile([128, A], F32, tag="proj", name="pk")
                for dt in range(8):
                    nc.tensor.matmul(
                        pk, wk_sb[:, dt, ct * 128:(ct + 1) * 128],
                        at_sb[:, dt, :], start=(dt == 0), stop=(dt == 7))
                nc.vector.tensor_scalar_add(
                    kt_sb[:, ct, :], pk, bk_sb[:, ct:ct + 1])

            # ---- V projection (un-transposed): v[a, c] = anchors @ Wv ----
            for ach in range(4):
                for ch in range(2):
                    pv = psum.tile([128, 512], F32, tag="proj", name="pv")
                    for dt in range(8):
                        nc.tensor.matmul(
                            pv, at_sb[:, dt, ach * 128:(ach + 1) * 128],
                            wv_sb[:, dt, ch * 512:(ch + 1) * 512],
                            start=(dt == 0), stop=(dt == 7))
                    # scatter heads ch*8..ch*8+7 into vaug slots
                    pv_v = pv.rearrange("p (hd d) -> p hd d", d=DH)
                    bv_v = bv_bc.rearrange("p (chd hd d) -> p chd hd d",
                                           chd=2, d=DH)[:, ch]
                    nc.vector.tensor_add(
                        vaug[:, ach, ch * 8:(ch + 1) * 8, 0:DH],
                        pv_v, bv_v)

            # ---- Q^T projection per 512-row chunk ----
            qts = []
            for rc in range(2):
                wsel = wlo_sb if rc == 0 else whi_sb
                bsel = blo_sb if rc == 0 else bhi_sb
                qt_sb = qtpool.tile([128, 8, 512], BF16, tag="qt",
                                    name=f"qt_sb{rc}")
                for ct in range(8):
                    pq = psum.tile([128, 512], F32, tag="proj", name="pq")
                    for dt in range(8):
                        nc.tensor.matmul(
                            pq, wsel[:, dt, ct * 128:(ct + 1) * 128],
                            xt_sb[:, dt, rc * 512:(rc + 1) * 512],
                            start=(dt == 0), stop=(dt == 7))
                    nc.vector.tensor_scalar_add(
                        qt_sb[:, ct, :], pq, bsel[:, ct:ct + 1])
                qts.append(qt_sb)

            # ---- attention: head pairs interleaved across both row chunks
            # so one pair's normalization chain overlaps the other's score
            # matmuls ----
            attnT = attnpool.tile([128, 8, RPC], BF16, name="attnT")
            for ct in range(8):            # head pair (2*ct, 2*ct+1)
                for rc in range(2):
                    qt_sb = qts[rc]
                    for par in range(2):
                        h = 2 * ct + par
                        po = par * 64      # partition offset of this head
                        # scores S^T[a, r] = K @ Q^T, per 128-anchor chunk
                        pt = ptpool.tile([128, 4, 512], BF16, tag="pt",
                                         name="pt")
                        for ach in range(4):
                            ps_s = psum.tile([128, 512], F32, tag="s",
                                             name="ps_s", bufs=3)
                            nc.tensor.matmul(
                                ps_s,
                                kt_sb[po:po + 64, ct, ach * 128:(ach + 1) * 128],
                                qt_sb[po:po + 64, ct, :], start=True, stop=True)
                            nc.scalar.activation(
                                out=pt[:, ach, :], in_=ps_s, func=Exp,
                                scale=SCALE)
                        # AV: [attn^T; sums] via ones-augmented V, rows 0-64
                        pav = psum.tile([128, 512], F32, tag="attn",
                                        name="pav", bufs=3)
                        for ach in range(4):
                            nc.tensor.matmul(
                                pav[0:DH + 1, :], vaug[:, ach, h, :],
                                pt[:, ach, :], start=(ach == 0),
                                stop=(ach == 3))
                        # normalization: sums row to partition 0 (single-
                        # partition DVE copies can cross partitions), then
                        # fast reciprocal (needs partition-0 SBUF input),
                        # partition-broadcast via free-dim stride-0 DMA,
                        # multiply out of PSUM.
                        srow = rcppool.tile([1, 512], F32, tag="srow",
                                            name="srow")
                        nc.vector.tensor_copy(srow, pav[DH:DH + 1, :])
                        rcp1 = rcppool.tile([1, 512], F32, tag="rcp",
                                            name="rcp1")
                        nc.vector.reciprocal_approx_fast(rcp1, srow)
                        bc_t = tmppool.tile([DH, 512], F32, tag="bc",
                                            name="bc_t")
                        nc.sync.dma_start(
                            out=bc_t,
                            in_=bass.AP(tensor=rcp1.tensor, offset=rcp1.offset,
                                        ap=[list(rcp1.ap)[0], [0, DH]]
                                        + list(rcp1.ap)[1:]))
                        if par == 0:
                            nc.vector.tensor_mul(
                                attnT[0:DH, ct, rc * 512:(rc + 1) * 512],
                                pav[0:DH, :], bc_t)
                        else:
                            # odd heads live at partitions 64-127 of the
                            # slab; DVE can't cross partitions, so stage +
                            # DMA-shift.
                            stg = tmppool.tile([DH, 512], BF16, tag="stg",
                                               name="stg")
                            nc.vector.tensor_mul(stg, pav[0:DH, :], bc_t)
                            nc.sync.dma_start(
                                out=attnT[DH:2 * DH, ct,
                                          rc * 512:(rc + 1) * 512],
                                in_=stg)

            # ---- output projection ----
            for rti in range(8):
                for nh in range(2):
                    pout = psum.tile([128, 512], F32, tag="proj",
                                     name="pout")
                    for ct2 in range(8):
                        nc.tensor.matmul(
                            pout, attnT[:, ct2, rti * 128:(rti + 1) * 128],
                            wo_sb[:, ct2, nh * 512:(nh + 1) * 512],
                            start=(ct2 == 0), stop=(ct2 == 7))
                    out_t = outpool.tile([128, 512], F32, tag="out",
                                         name="out_t")
                    nc.vector.tensor_add(out_t, pout,
                                         bo_bc[:, nh * 512:(nh + 1) * 512])
                    nc.sync.dma_start(
                        out=out.ap()[rti * 128:(rti + 1) * 128,
                                     nh * 512:(nh + 1) * 512],
                        in_=out_t)

    nc.compile()
    return nc


def _make_in_maps(x, Wq, bq, Wk, bk, Wv, bv, Wqt, bqt, Wo, bo):
    x = np.asarray(x, dtype=np.float32)
    bf = ml_dtypes.bfloat16

    wq_b = np.ascontiguousarray(np.asarray(Wq, np.float32).astype(bf))
    wqt_b = np.ascontiguousarray(np.asarray(Wqt, np.float32).astype(bf))
    wk_b = np.ascontiguousarray(np.asarray(Wk, np.float32).astype(bf))
    wv_b = np.ascontiguousarray(np.asarray(Wv, np.float32).astype(bf))
    wo_b = np.ascontiguousarray(np.asarray(Wo, np.float32).astype(bf))
    f32 = lambda v: np.ascontiguousarray(np.asarray(v, np.float32))
    bq, bqt, bk, bv, bo = map(f32, (bq, bqt, bk, bv, bo))

    in_maps = []
    for c in range(NCORES):
        b, q = divmod(c, 4)
        rows = x[b, q * RPC:(q + 1) * RPC, :]
        in_maps.append({
            "xt": np.ascontiguousarray(rows.T.astype(bf)),
            "at": np.ascontiguousarray(x[b, :A, :].T.astype(bf)),
            "wlo": wq_b if q == 0 else wqt_b,
            "whi": wqt_b,
            "wk": wk_b, "wv": wv_b, "wo": wo_b,
            "blo": bq if q == 0 else bqt, "bhi": bqt,
            "bk": bk, "bv": bv, "bo": bo,
        })
    return in_maps


def kernel(x, Wq, bq, Wk, bk, Wv, bv, Wqt, bqt, Wo, bo, num_anchor_tokens):
    assert int(num_anchor_tokens) == A
    if "nc" not in _CACHE:
        _CACHE["nc"] = _build()
    nc = _CACHE["nc"]

    in_maps = _make_in_maps(x, Wq, bq, Wk, bk, Wv, bv, Wqt, bqt, Wo, bo)
    res = bass_utils.run_bass_kernel_spmd(
        nc, in_maps, core_ids=list(range(NCORES)))
    out = np.empty((B, S, D), np.float32)
    for c in range(NCORES):
        b, q = divmod(c, 4)
        out[b, q * RPC:(q + 1) * RPC, :] = res.results[c]["out"]
    return out
